# revision 18
# baseline (speedup 1.0000x reference)
"""Multi-head attention (B=2, T=2048, D=2048, H=16, HD=128) on 8 Trainium2
NeuronCores.

Sharding: core c handles batch b = c // 4 and head group g = c % 4 (4 heads
per core). wq/wk/wv column-sharded, wo row-sharded; partial outputs summed on
the host, batches stacked.

Device kernel (per core, all-bf16 data path, f32 PSUM accumulation):
  pass 1: per 512-wide x quarter, K projection (roped) and V, k-outer loop
          (4 parallel PSUM accumulators) so the first weight DMA chunk feeds
          PE within ~3us; quarter 0 additionally projects Q(quarter 0).
  pass 2: software-pipelined slots. Slot (qc, h) interleaves, per k-tile:
            - scoresT matmul for combo cc=(qc,h) (KT stationary, QT moving)
            - exp on ACT into bf16 pt
            - DVE running sum of exp tiles (softmax denominator; replaces
              the ones-matmul stream that cost ~55us of PE in v1)
            - attention-out matmul for combo cc-1 (V stationary, pt moving)
            - Q-projection matmul for quarter qc+1, head h
            - every few k: one wo-projection group of quarter qc-1, with the
              PSUM->SBUF copy on the otherwise idle GpSimd engine
          Denominator -> reciprocal -> broadcast matmul -> DVE normalize as
          a 4-stage pipeline spread over the next slot's k-loop.
PSUM: sc(2) + ou(2) + qacc(2) + misc(2) = 8 banks.
"""
from contextlib import ExitStack

import numpy as np

B, T, D, H = 2, 2048, 2048, 16
HD = D // H            # 128
N_CORES = 8
HPC = H // 4           # 4 heads per core
JC = HPC * HD          # 512 per-core projection width
KD = D // 128          # 16 contraction tiles
TQ = 512               # t-quarter width
N_TQ = T // TQ         # 4
QC = 512               # q-chunk width (== TQ)
KT_TILES = T // 128    # 16

import os as _os

PHASES = _os.environ.get("K_PHASES", "full")  # p1 | full

_cache = {}


def _build_program():
    import concourse.bacc as bacc
    import concourse.tile as tile
    from concourse import mybir

    F32 = mybir.dt.float32
    BF16 = mybir.dt.bfloat16
    AF = mybir.ActivationFunctionType
    ALU = mybir.AluOpType

    nc = bacc.Bacc("TRN2", target_bir_lowering=False, debug=False,
                   num_devices=N_CORES)

    xq = nc.dram_tensor("xq", [128, N_TQ * KD * TQ], BF16,
                        kind="ExternalInput").ap()
    wkb = nc.dram_tensor("wkb", [128, KD * JC], BF16,
                         kind="ExternalInput").ap()
    wvb = nc.dram_tensor("wvb", [128, KD * JC], BF16,
                         kind="ExternalInput").ap()
    wqb = nc.dram_tensor("wqb", [128, KD * JC], BF16,
                         kind="ExternalInput").ap()
    wob = nc.dram_tensor("wob", [128, HPC * D], BF16,
                         kind="ExternalInput").ap()
    csA = nc.dram_tensor("csA", [128, T], F32, kind="ExternalInput").ap()
    csB = nc.dram_tensor("csB", [128, T], F32, kind="ExternalInput").ap()
    py = nc.dram_tensor("py", [T, D], BF16,
                        kind="ExternalOutput").ap()

    with tile.TileContext(nc) as tc, ExitStack() as ctx:
        # persistent tiles on the right side of the SBUF stack
        p_big = ctx.enter_context(tc.tile_pool(name="big", bufs=1,
                                               side="right"))
        KT = [[p_big.tile([128, TQ], BF16, tag=f"KT{h}_{q}",
                          name=f"KT{h}_{q}") for q in range(N_TQ)]
              for h in range(HPC)]
        QT = [[p_big.tile([128, TQ], BF16, tag=f"QT{h}_{q}",
                          name=f"QT{h}_{q}") for q in range(N_TQ)]
              for h in range(HPC)]
        V = [p_big.tile([128, JC], BF16, tag=f"V{t}", name=f"V{t}")
             for t in range(KT_TILES)]
        csa_t = p_big.tile([128, T], F32, tag="csa")
        csb_t = p_big.tile([128, T], F32, tag="csb")
        onesKK = p_big.tile([128, 128], BF16, tag="onesKK")
        wq_t = p_big.tile([128, KD * JC], BF16, tag="wq", name="wq")
        wo_t = p_big.tile([128, HPC * D], BF16, tag="wo", name="wo")
        p_x2 = ctx.enter_context(tc.tile_pool(name="x2", bufs=8,
                                              side="right"))
        xc = {q: [None] * 4 for q in range(1, N_TQ)}

        def xchunk_load(q, c):
            t = p_x2.tile([128, 2048], BF16, tag="xc", name="xc")
            nc.sync.dma_start(
                t[:], xq[:, q * KD * TQ + c * 2048:q * KD * TQ +
                         (c + 1) * 2048])
            xc[q][c] = t

        def rope(p_rt, ps, dst, qs):
            """dst = rotate(ps) using csA/csB tables; dst may be bf16."""
            u = p_rt.tile([128, TQ], F32, tag="ru", name="ru")
            v = p_rt.tile([128, TQ], F32, tag="rv", name="rv")
            nc.vector.tensor_tensor(u[:], ps[:], csa_t[:, qs:qs + TQ],
                                    ALU.mult)
            nc.vector.tensor_tensor(v[0:64, :], ps[64:128, :],
                                    csb_t[0:64, qs:qs + TQ], ALU.mult)
            nc.vector.tensor_tensor(v[64:128, :], ps[0:64, :],
                                    csb_t[64:128, qs:qs + TQ], ALU.mult)
            nc.vector.tensor_tensor(dst, u[:], v[:], ALU.add)

        # ---------------- pass 1: K + V (+ Q of quarter 0) ----------------
        with tc.tile_pool(name="w1", bufs=1) as p_w, \
             tc.tile_pool(name="x1", bufs=2) as p_x, \
             tc.tile_pool(name="rt1", bufs=3) as p_rt, \
             tc.tile_pool(name="psA", bufs=4, space="PSUM") as psA:

            def load_xte(pool, q, interleave=None):
                t = pool.tile([128, KD * TQ], BF16, tag="xte", name="xte")
                for c in range(4):
                    nc.sync.dma_start(
                        t[:, c * 2048:(c + 1) * 2048],
                        xq[:, q * KD * TQ + c * 2048:
                           q * KD * TQ + (c + 1) * 2048])
                    if interleave is not None:
                        interleave(c)
                return t

            wk_t = p_w.tile([128, KD * JC], BF16, tag="wk", name="wk")
            wv_t = p_w.tile([128, KD * JC], BF16, tag="wv", name="wv")

            def wk_chunk(c):
                nc.sync.dma_start(wk_t[:, c * 2048:(c + 1) * 2048],
                                  wkb[:, c * 2048:(c + 1) * 2048])

            # DMA order: (xte0 | wk) interleaved, cs q0, wv, wq, cs q1-3,
            # ones, wo.  The SP queue drains roughly in order; the first
            # chunk is split per k-slice so the first K matmul starts after
            # ~0.25MB.
            xte = p_x.tile([128, KD * TQ], BF16, tag="xte", name="xte")
            for s in range(4):
                nc.sync.dma_start(xte[:, s * 512:(s + 1) * 512],
                                  xq[:, s * 512:(s + 1) * 512])
                nc.sync.dma_start(wk_t[:, s * 512:(s + 1) * 512],
                                  wkb[:, s * 512:(s + 1) * 512])
            for c in range(1, 4):
                nc.sync.dma_start(
                    xte[:, c * 2048:(c + 1) * 2048],
                    xq[:, c * 2048:(c + 1) * 2048])
                wk_chunk(c)
            nc.sync.dma_start(csa_t[:, 0:TQ], csA[:, 0:TQ])
            nc.sync.dma_start(csb_t[:, 0:TQ], csB[:, 0:TQ])
            for c in range(4):
                nc.sync.dma_start(wv_t[:, c * 2048:(c + 1) * 2048],
                                  wvb[:, c * 2048:(c + 1) * 2048])
            for c in range(4):
                nc.sync.dma_start(wq_t[:, c * 2048:(c + 1) * 2048],
                                  wqb[:, c * 2048:(c + 1) * 2048])
            for q in range(1, 4):
                nc.sync.dma_start(csa_t[:, q * TQ:(q + 1) * TQ],
                                  csA[:, q * TQ:(q + 1) * TQ])
                nc.sync.dma_start(csb_t[:, q * TQ:(q + 1) * TQ],
                                  csB[:, q * TQ:(q + 1) * TQ])
            nc.gpsimd.memset(onesKK[:], 1.0)
            for c in range(8):
                nc.sync.dma_start(wo_t[:, c * 1024:(c + 1) * 1024],
                                  wob[:, c * 1024:(c + 1) * 1024])

            # prime the ACT exp table while PE is busy with projections
            dummy = p_rt.tile([1, 2], F32, tag="prime", name="prime")
            nc.scalar.activation(dummy[:], csa_t[0:1, 0:2], AF.Exp)

            def proj_w_stationary(wt, xt, dsts, qs):
                """K/Q projection, k-outer: 4 parallel accumulators."""
                accs = [psA.tile([128, TQ], F32, tag="pk", name=f"pk{_j}")
                        for _j in range(4)]
                for k in range(KD):
                    for j in range(HPC):
                        nc.tensor.matmul(
                            accs[j][:],
                            wt[:, k * JC + j * 128:k * JC + (j + 1) * 128],
                            xt[:, k * TQ:(k + 1) * TQ],
                            start=(k == 0), stop=(k == KD - 1))
                for j in range(HPC):
                    rope(p_rt, accs[j], dsts[j][qs // TQ][:], qs)

            for q in range(N_TQ):
                if q > 0:
                    xte_next = xte
                    xte = load_xte(p_x, q)
                proj_w_stationary(wk_t, xte, KT, q * TQ)
                # V: x stationary, k-inner per t-tile so the PSUM->SBUF
                # copy of each tile lands right away (shortens the pool
                # release tail at the pass boundary)
                for tl in range(4):
                    acc = psA.tile([128, JC], F32, tag="pv", name="pv")
                    for k in range(KD):
                        nc.tensor.matmul(
                            acc[:],
                            xte[:, k * TQ + tl * 128:k * TQ + (tl + 1) * 128],
                            wv_t[:, k * JC:(k + 1) * JC],
                            start=(k == 0), stop=(k == KD - 1))
                    nc.scalar.copy(V[q * 4 + tl][:], acc[:])
                if q == 0:
                    proj_w_stationary(wq_t, xte, QT, 0)
            for c in range(4):
                xchunk_load(1, c)

        # ---------------- pass 2: pipelined attention + output ------------
        if PHASES == "p1":
            nc.compile()
            return nc

        with tc.tile_pool(name="pt", bufs=2) as p_pt, \
             tc.tile_pool(name="sS", bufs=2) as p_s, \
             tc.tile_pool(name="ao", bufs=8) as p_ao, \
             tc.tile_pool(name="rt2", bufs=4) as p_rt2, \
             tc.tile_pool(name="bm", bufs=4) as p_bm, \
             tc.tile_pool(name="po", bufs=6) as p_po, \
             tc.tile_pool(name="psSC", bufs=3, space="PSUM") as psSC, \
             tc.tile_pool(name="psOU", bufs=2, space="PSUM") as psOU, \
             tc.tile_pool(name="psQ", bufs=2, space="PSUM") as psQ, \
             tc.tile_pool(name="psBC", bufs=1, space="PSUM") as psBC:

            # state carried between slots
            state = {}
            ao_q = {q: [None] * HPC for q in range(N_TQ)}

            c_alt = [0]

            def c_mm(acc, cq, tl, ec, j):
                nc.tensor.matmul(
                    acc[:],
                    ao_q[cq][j][:, tl * 128:(tl + 1) * 128],
                    wo_t[:, j * D + ec * 512:j * D + (ec + 1) * 512],
                    start=(j == 0), stop=(j == HPC - 1))

            def c_finish(acc, cq, tl, ec):
                out_sb = p_po.tile([128, 512], BF16, tag="po", name="po")
                c_alt[0] ^= 1
                if c_alt[0]:
                    nc.scalar.copy(out_sb[:], acc[:])
                else:
                    nc.vector.tensor_copy(out_sb[:], acc[:])
                ts = cq * TQ + tl * 128
                nc.sync.dma_start(py[ts:ts + 128, ec * 512:(ec + 1) * 512],
                                  out_sb[:])

            def c_group(cq, tl, ec, pool=None, tag="sc"):
                """One wo-projection group: py[cq, tl, ec] over 4 heads."""
                acc = (pool or psSC).tile([128, 512], F32, tag=tag,
                                          name="cacc")
                for j in range(HPC):
                    c_mm(acc, cq, tl, ec, j)
                c_finish(acc, cq, tl, ec)

            class CStream:
                """Emit C-group matmuls spread ~evenly over the k loop,
                with accumulators from an otherwise idle PSUM pool."""

                def __init__(self, items, pool, tag):
                    self.mms = [(it, j) for it in items for j in range(HPC)]
                    self.pool, self.tag = pool, tag
                    self.pos = 0
                    self.acc = None

                def step(self, k):
                    n = len(self.mms)
                    take = ((k + 1) * n) // KT_TILES - (k * n) // KT_TILES
                    for _ in range(take):
                        it, j = self.mms[self.pos]
                        if j == 0:
                            self.acc = self.pool.tile(
                                [128, 512], F32, tag=self.tag, name="cacc")
                        c_mm(self.acc, *it, j)
                        if j == HPC - 1:
                            c_finish(self.acc, *it)
                        self.pos += 1

            def slot(cc):
                qc, h = divmod(cc, 4)
                live = cc < 16          # this slot starts a new combo
                prev = cc - 1 if cc >= 1 else None
                qn = qc + 1             # quarter whose Q we project
                do_q = live and qn < N_TQ

                if live and cc < 8:     # prefetch x chunks 2 quarters out
                    xchunk_load(2 + cc // 4, cc % 4)

                # C work: quarter cq's groups in the 4 slots after ao(cq)
                # completes.  qc<3 slots use psSC blobs; qc==3 slots (no
                # q-projection filler) use the idle psQ pool, spread 1 matmul
                # per k so PE outpaces the ACT exp stream.
                cwork, cstream = [], None

                def citems(cq, lo, hi):
                    return [(cq, i // 4, i % 4) for i in range(lo, hi)]

                if live:
                    if qc in (1, 2) and h >= 1:
                        lo, hi = ([(0, 6), (6, 11), (11, 16)] if qc == 1
                                  else [(0, 4), (4, 8), (8, 12)])[h - 1]
                        cwork = citems(qc - 1, lo, hi)
                    elif qc == 3:
                        items = (citems(1, 12, 16) if h == 0 else
                                 citems(2, *[(0, 6), (6, 11),
                                             (11, 16)][h - 1]))
                        cstream = CStream(items, psQ, "qacc")

                if live:
                    pt_cur = p_pt.tile([128, KT_TILES * QC], BF16, tag="pt", name="ptc")
                    S_cur = p_s.tile([128, QC], BF16, tag="S", name="Sc")
                    if do_q:
                        qacc = psQ.tile([128, TQ], F32, tag="qacc", name="qacc")
                qs = qc * QC

                if prev is not None:
                    pq, ph = divmod(prev, 4)
                    pt_prev = state["pt"]
                    S_prev = state["S"]
                    ou_prev = psOU.tile([128, QC], F32, tag="ou", name="ou")

                ci = 0
                for k in range(KT_TILES):
                    if live:
                        sc = psSC.tile([128, QC], F32, tag="sc", name="sct")
                        nc.tensor.matmul(
                            sc[:],
                            KT[h][k // 4][:, (k % 4) * 128:(k % 4 + 1) * 128],
                            QT[h][qc][:], start=True, stop=True)
                        nc.scalar.activation(
                            pt_cur[:, k * QC:(k + 1) * QC], sc[:], AF.Exp)
                        if k == 0:
                            nc.vector.tensor_copy(S_cur[:], pt_cur[:, 0:QC])
                        else:
                            nc.vector.tensor_tensor(
                                S_cur[:], S_cur[:],
                                pt_cur[:, k * QC:(k + 1) * QC], ALU.add)
                    if prev is not None:
                        nc.tensor.matmul(
                            ou_prev[:], V[k][:, ph * 128:(ph + 1) * 128],
                            pt_prev[:, k * QC:(k + 1) * QC],
                            start=(k == 0), stop=(k == KT_TILES - 1))
                        # normalization pipeline for prev, spread over k
                        if k == 2:
                            bcd = psBC.tile([128, QC], F32, tag="bcd",
                                            name="bcd")
                            nc.tensor.matmul(bcd[:], onesKK[:], S_prev[:],
                                             start=True, stop=True)
                        elif k == 4:
                            rc_sb = p_bm.tile([128, QC], BF16, tag="rc",
                                              name="rcsb")
                            with nc.allow_low_precision(
                                    reason="softmax denom bf16"):
                                nc.vector.reciprocal(rc_sb[:], bcd[:])
                    if do_q:
                        nc.tensor.matmul(
                            qacc[:],
                            wq_t[:, k * JC + h * 128:k * JC + (h + 1) * 128],
                            xc[qn][k // 4][:, (k % 4) * TQ:
                                           (k % 4 + 1) * TQ],
                            start=(k == 0), stop=(k == KT_TILES - 1))
                    if cwork and k in (2, 5, 8, 11, 14, 15):
                        if ci < len(cwork):
                            c_group(*cwork[ci])
                            ci += 1
                    if cstream is not None:
                        cstream.step(k)
                while ci < len(cwork):
                    c_group(*cwork[ci])
                    ci += 1

                if prev is not None:
                    ao_h = p_ao.tile([128, QC], BF16, tag="ao", name="aot")
                    nc.vector.tensor_tensor(ao_h[:], ou_prev[:], rc_sb[:],
                                            ALU.mult)
                    ao_q[pq][ph] = ao_h
                if do_q:
                    rope(p_rt2, qacc, QT[h][qn][:], qn * TQ)
                if live:
                    state["pt"] = pt_cur
                    state["S"] = S_cur

            for cc in range(17):
                slot(cc)
            # trailing C for quarter 3
            for i in range(16):
                c_group(3, i // 4, i % 4)

    nc.compile()
    return nc


def _to_bf16(a):
    import ml_dtypes
    return np.asarray(a, dtype=np.float32).astype(ml_dtypes.bfloat16)


def _prep_inputs(x, freqs_cis, wq, wk, wv, wo):
    """Host-side shard + layout prep. Returns in_maps for the 8 cores."""
    scale = HD ** (-0.5)
    perm = np.concatenate([np.arange(0, HD, 2), np.arange(1, HD, 2)])

    cos = np.ascontiguousarray(freqs_cis[:, :, 0].T, dtype=np.float32)
    sin = np.ascontiguousarray(freqs_cis[:, :, 1].T, dtype=np.float32)
    csA = np.concatenate([cos, cos], axis=0)          # (128, T)
    csB = np.concatenate([-sin, sin], axis=0)         # (128, T)

    def wlay(wT):  # (D, JC) -> (128, KD*JC), k-major contiguous
        return np.ascontiguousarray(
            wT.reshape(KD, 128, JC).transpose(1, 0, 2).reshape(128, KD * JC))

    in_maps = []
    xq_cache = {}
    for c in range(N_CORES):
        b, g = divmod(c, 4)
        rows = slice(g * JC, (g + 1) * JC)
        wq_g = wq[rows].reshape(HPC, HD, D)[:, perm].reshape(JC, D) * scale
        wk_g = wk[rows].reshape(HPC, HD, D)[:, perm].reshape(JC, D)
        wv_g = wv[rows]
        wo_g = wo[:, rows]
        if b not in xq_cache:
            xT = np.ascontiguousarray(x[b].T)  # (D, T)
            xq_cache[b] = _to_bf16(
                xT.reshape(KD, 128, N_TQ, TQ).transpose(1, 2, 0, 3)
                .reshape(128, N_TQ * KD * TQ))
        woT = wo_g.T  # (JC, D)
        in_maps.append({
            "xq": xq_cache[b],
            "wkb": _to_bf16(wlay(wk_g.T)),
            "wvb": _to_bf16(wlay(wv_g.T)),
            "wqb": _to_bf16(wlay(wq_g.T)),
            "wob": _to_bf16(
                woT.reshape(HPC, 128, D).transpose(1, 0, 2)
                .reshape(128, HPC * D)),
            "csA": csA,
            "csB": csB,
        })
    return in_maps


def _make_runner(nc):
    """Cacheable jitted SPMD runner."""
    import jax
    from concourse import mybir
    from concourse.bass2jax import (
        _bass_exec_p, install_neuronx_cc_hook, partition_id_tensor)
    from jax.experimental.shard_map import shard_map
    from jax.sharding import Mesh, NamedSharding, PartitionSpec

    install_neuronx_cc_hook()
    partition_name = (
        nc.partition_id_tensor.name if nc.partition_id_tensor else None)
    in_names, out_names, out_avals, zero_outs = [], [], [], []
    for alloc in nc.m.functions[0].allocations:
        if not isinstance(alloc, mybir.MemoryLocationSet):
            continue
        name = alloc.memorylocations[0].name
        if alloc.kind == "ExternalInput":
            if name != partition_name:
                in_names.append(name)
        elif alloc.kind == "ExternalOutput":
            out_names.append(name)
            shape = tuple(alloc.tensor_shape)
            dtype = mybir.dt.np(alloc.dtype)
            out_avals.append(jax.core.ShapedArray(shape, dtype))
            zero_outs.append(np.zeros(shape, dtype))
    all_in_names = list(in_names) + out_names
    if partition_name is not None:
        all_in_names.append(partition_name)

    def _body(*args):
        operands = list(args)
        if partition_name is not None:
            operands.append(partition_id_tensor())
        outs = _bass_exec_p.bind(
            *operands,
            out_avals=tuple(out_avals),
            in_names=tuple(all_in_names),
            out_names=tuple(out_names),
            lowering_input_output_aliases=(),
            sim_require_finite=True,
            sim_require_nnan=True,
            nc=nc,
        )
        return tuple(outs)

    devices = jax.devices()[:N_CORES]
    assert len(devices) == N_CORES, f"need {N_CORES} devices, got {devices}"
    mesh = Mesh(np.asarray(devices), ("core",))
    nshard = NamedSharding(mesh, PartitionSpec("core"))
    n_in = len(in_names) + len(out_names)
    jf = jax.jit(
        shard_map(_body, mesh=mesh,
                  in_specs=(PartitionSpec("core"),) * n_in,
                  out_specs=(PartitionSpec("core"),) * len(out_names),
                  check_rep=False),
        keep_unused=True,
    )
    dev_zero = [
        jax.device_put(
            np.zeros((N_CORES * z.shape[0], *z.shape[1:]), z.dtype), nshard)
        for z in zero_outs
    ]

    def run(in_maps):
        concat_in = [
            np.concatenate([np.asarray(in_maps[c][nm])
                            for c in range(N_CORES)], axis=0)
            for nm in in_names
        ]
        dev_in = [jax.device_put(a, nshard) for a in concat_in]
        outs = jf(*dev_in, *dev_zero)
        return {
            name: np.asarray(outs[i]) for i, name in enumerate(out_names)
        }

    return run


def kernel(x, freqs_cis, wq, wk, wv, wo):
    if "nc" not in _cache:
        _cache["nc"] = _build_program()
    if "run" not in _cache:
        _cache["run"] = _make_runner(_cache["nc"])

    in_maps = _prep_inputs(
        np.asarray(x), np.asarray(freqs_cis), np.asarray(wq),
        np.asarray(wk), np.asarray(wv), np.asarray(wo))
    outs = _cache["run"](in_maps)
    pys = outs["py"].reshape(N_CORES, T, D)

    out = np.empty((B, T, D), dtype=np.float32)
    for b in range(B):
        acc = pys[b * 4].astype(np.float64)
        for g in range(1, 4):
            acc += pys[b * 4 + g].astype(np.float64)
        out[b] = acc.astype(np.float32)
    return out
